# revision 19
# baseline (speedup 1.0000x reference)
"""Trainium2 Bass kernel for BaselineAttention (B=4, S=2048, D=1024, H=8).

Sharding: 8 cores = 4 batches x 2 head-groups (4 heads each).
Each core computes, for its (batch, 4 heads):
  q/k/v projections (bf16 matmuls, fp32 accum), causal attention with
  fp32 softmax (exp on ACT with fused row-sums), attention-weight output,
  and its row-parallel slice of the final fc (partials summed on host).

Inputs/weights are cast to bf16 on the host (part of the sharding/layout
step); transposed activation layouts are produced by the DMA xbar
transpose directly from DRAM. Logits are computed in both [sq,sk] and
[sk,sq] orientations on the TensorEngine (cheaper than transposing the
softmax output on-chip); q/k head pairs share the PE array via row
groups (K=64 each).
"""

import math
import os
from contextlib import ExitStack

import ml_dtypes
import numpy as np

import concourse.bass as bass
import concourse.mybir as mybir
import concourse.tile as tile
from concourse import bacc
from concourse.bass_utils import run_bass_kernel_spmd
from concourse.masks import make_causal_mask, make_identity

B, S, D, H = 4, 2048, 1024, 8
DH = D // H            # 128 (v head dim)
DQK = (D // 2) // H    # 64  (q/k head dim)
HPC = H // 2           # 4 heads per core
NBLK = S // 128        # 16
SCALE = 1.0 / math.sqrt(DH)
NEG = -1e9

bf16 = mybir.dt.bfloat16
f32 = mybir.dt.float32
AF = mybir.ActivationFunctionType

_CACHE = {}
last_result = None  # stashed BassKernelResults when KERNEL_TRACE=1


def _to_bf16(x):
    """Round-to-nearest-even float32 -> bfloat16, numpy only."""
    u = np.ascontiguousarray(x, dtype=np.float32).view(np.uint32)
    r = (u >> 16) & np.uint32(1)
    out = ((u + np.uint32(0x7FFF) + r) >> 16).astype(np.uint16)
    return out.view(ml_dtypes.bfloat16)


def _tri_offsets(causal):
    """Column offsets of each sk-chunk's row inside the triangular expT tile."""
    offs, tot = [], 0
    for c in range(NBLK):
        offs.append(tot)
        tot += (S - c * 128) if causal else S
    return offs, tot


def _build(causal: bool):
    nc = bacc.Bacc("TRN2", target_bir_lowering=False, debug=False)

    q_in = nc.dram_tensor("q_in", [S, D], bf16, kind="ExternalInput").ap()
    k_in = nc.dram_tensor("k_in", [S, D], bf16, kind="ExternalInput").ap()
    v_in = nc.dram_tensor("v_in", [S, D], bf16, kind="ExternalInput").ap()
    # weights pre-arranged host-side: [partition, k-chunk, out-cols]
    wq = nc.dram_tensor("wq", [128, 8, 2 * DH], bf16, kind="ExternalInput").ap()
    wk = nc.dram_tensor("wk", [128, 8, 2 * DH], bf16, kind="ExternalInput").ap()
    wv = nc.dram_tensor("wv", [128, 8, 4 * DH], bf16, kind="ExternalInput").ap()
    wfc = nc.dram_tensor("wfc", [128, HPC, D], bf16, kind="ExternalInput").ap()
    # bias rows (added via K=1 rank-1 matmuls into the psum accumulation)
    bq = nc.dram_tensor("bq", [1, 2 * DH], bf16, kind="ExternalInput").ap()
    bk = nc.dram_tensor("bk", [1, 2 * DH], bf16, kind="ExternalInput").ap()
    bv = nc.dram_tensor("bv", [1, 4 * DH], bf16, kind="ExternalInput").ap()
    bfc = nc.dram_tensor("bfc", [1, D], bf16, kind="ExternalInput").ap()

    rscr = nc.dram_tensor("rscr", [HPC, S], f32).ap()
    attn_w = nc.dram_tensor("attn_w", [HPC, S, S], f32, kind="ExternalOutput").ap()
    out_p = nc.dram_tensor("out_p", [S, D], f32, kind="ExternalOutput").ap()

    TRI_OFF, TRI = _tri_offsets(causal)

    with tile.TileContext(nc) as tc, ExitStack() as ctx:
        singles = ctx.enter_context(tc.tile_pool(name="singles", bufs=1))
        ident = singles.tile([128, 128], bf16)
        make_identity(nc, ident)
        if causal:
            maskN = singles.tile([128, 128], bf16)
            make_causal_mask(nc, maskN, mask_val=NEG)  # fill where col > row
            maskT = singles.tile([128, 128], bf16)     # fill where row > col
            nc.gpsimd.memset(maskT, 0.0)
            nc.gpsimd.affine_select(
                out=maskT, in_=maskT, compare_op=mybir.AluOpType.is_ge,
                fill=NEG, base=0, pattern=[[1, 128]], channel_multiplier=-1,
            )
        # weights via SWDGE so the HWDGE xbar stays in transpose mode
        wq_sb = singles.tile([128, 8, 2 * DH], bf16)
        nc.gpsimd.dma_start(out=wq_sb, in_=wq)
        wk_sb = singles.tile([128, 8, 2 * DH], bf16)
        nc.gpsimd.dma_start(out=wk_sb, in_=wk)
        wv_sb = singles.tile([128, 8, 4 * DH], bf16)
        nc.gpsimd.dma_start(out=wv_sb, in_=wv)
        wfc_sb = singles.tile([128, HPC, D], bf16)
        nc.gpsimd.dma_start(out=wfc_sb, in_=wfc)
        bq_sb = singles.tile([1, 2 * DH], bf16)
        nc.gpsimd.dma_start(out=bq_sb, in_=bq)
        bk_sb = singles.tile([1, 2 * DH], bf16)
        nc.gpsimd.dma_start(out=bk_sb, in_=bk)
        bv_sb = singles.tile([1, 4 * DH], bf16)
        nc.gpsimd.dma_start(out=bv_sb, in_=bv)
        bfc_sb = singles.tile([1, D], bf16)
        nc.gpsimd.dma_start(out=bfc_sb, in_=bfc)
        ones1 = singles.tile([1, 512], bf16)
        nc.vector.memset(ones1, 1.0)

        acts = ctx.enter_context(tc.tile_pool(name="acts", bufs=1))
        qT = [acts.tile([128, S], bf16, tag=f"qT{i}", name=f"qT{i}")
              for i in range(2)]
        kT = [acts.tile([128, S], bf16, tag=f"kT{i}", name=f"kT{i}")
              for i in range(2)]
        vA = acts.tile([128, NBLK, 4 * DH], bf16, tag="v")
        attnT = acts.tile([128, HPC, S], bf16, tag="attnT")

        # ---- transpose-load inputs (DMA xbar from DRAM bf16), project ----
        with tc.tile_pool(name="xT", bufs=2) as xTp, \
             tc.tile_pool(name="projps", bufs=2, space="PSUM") as projps:

            def load_xT(x_dram):
                xT_t = xTp.tile([128, 8, S], bf16, tag="xT")
                nc.sync.dma_start(out=xT_t[:, 0:4, :], in_=x_dram[:, 0:512],
                                  transpose=True)
                nc.sync.dma_start(out=xT_t[:, 4:8, :], in_=x_dram[:, 512:1024],
                                  transpose=True)
                return xT_t

            for x_dram, w_sb, b_sb, dstT in ((q_in, wq_sb, bq_sb, qT),
                                             (k_in, wk_sb, bk_sb, kT)):
                xt = load_xT(x_dram)
                for pair in range(2):
                    for st in range(4):
                        ps = projps.tile([128, 512], f32, tag="pps")
                        for c in range(8):
                            nc.tensor.matmul(
                                ps, w_sb[:, c, pair * 128:(pair + 1) * 128],
                                xt[:, c, st * 512:(st + 1) * 512],
                                start=(c == 0), stop=False)
                        # bias: column (x) ones-row outer product
                        nc.tensor.matmul(
                            ps, b_sb[:, pair * 128:(pair + 1) * 128],
                            ones1, start=False, stop=True)
                        nc.vector.tensor_copy(
                            dstT[pair][:, st * 512:(st + 1) * 512], ps)
            xt = load_xT(v_in)
            for b in range(NBLK):
                ps = projps.tile([128, 512], f32, tag="pps")
                for c in range(8):
                    nc.tensor.matmul(ps, xt[:, c, b * 128:(b + 1) * 128],
                                     wv_sb[:, c, :], start=(c == 0), stop=False)
                nc.tensor.matmul(ps, ones1[:, 0:128], bv_sb,
                                 start=False, stop=True)
                nc.vector.tensor_copy(vA[:, b, :], ps)

        # ---- main attention pools ----
        lg_ps = ctx.enter_context(tc.tile_pool(name="lgps", bufs=3, space="PSUM"))
        at_ps = ctx.enter_context(tc.tile_pool(name="atps", bufs=2, space="PSUM"))
        expnat = ctx.enter_context(tc.tile_pool(name="expnat", bufs=4))
        expT_p = ctx.enter_context(tc.tile_pool(name="expT", bufs=1))
        aw_p = ctx.enter_context(tc.tile_pool(name="aw", bufs=3))
        small = ctx.enter_context(tc.tile_pool(name="small", bufs=6))
        rec_p = ctx.enter_context(tc.tile_pool(name="rec", bufs=2))
        rb_p = ctx.enter_context(tc.tile_pool(name="rb", bufs=3))
        out_sb = ctx.enter_context(tc.tile_pool(name="outsb", bufs=2))

        def emit_fc(sqb):
            ot = out_sb.tile([128, D], f32, tag="out", name="ot")
            for st in range(2):
                ps = at_ps.tile([128, 512], f32, tag="at", name="fps")
                for c in range(HPC):
                    nc.tensor.matmul(ps, attnT[:, c, sqb * 128:(sqb + 1) * 128],
                                     wfc_sb[:, c, st * 512:(st + 1) * 512],
                                     start=(c == 0), stop=False)
                nc.tensor.matmul(ps, ones1[:, 0:128],
                                 bfc_sb[:, st * 512:(st + 1) * 512],
                                 start=False, stop=True)
                nc.vector.tensor_copy(ot[:, st * 512:(st + 1) * 512], ps)
            nc.gpsimd.dma_start(out=out_p[sqb * 128:(sqb + 1) * 128, :], in_=ot)

        for pt in range(2):
            qs = [qT[pt][0:64, :], qT[pt][64:128, :]]
            ks = [kT[pt][0:64, :], kT[pt][64:128, :]]
            recs = [rec_p.tile([128, NBLK], f32, tag="rec", name=f"rec{s}")
                    for s in range(2)]

            # natural side, head pair packed into PE row groups
            for sqb in range(NBLK):
                sk_len = (sqb + 1) * 128 if causal else S
                ntile = (sk_len + 1023) // 1024
                ets = [expnat.tile([128, S], bf16, tag="expnat", name=f"et{s}")
                       for s in range(2)]
                parts = [small.tile([128, 2], f32, tag="parts", name=f"pt{s}")
                         for s in range(2)]
                for i in range(ntile):
                    tw = min(1024, sk_len - i * 1024)
                    pss = [lg_ps.tile([128, 1024], f32, tag="lg", name=f"lg{s}")
                           for s in range(2)]
                    pos = 0
                    while pos < tw:
                        w = min(512, tw - pos)
                        col = i * 1024 + pos
                        last = causal and (col + w == sk_len)
                        for s in range(2):
                            nc.tensor.matmul(
                                pss[s][:, pos:pos + w],
                                qs[s][:, sqb * 128:(sqb + 1) * 128],
                                ks[s][:, col:col + w],
                                start=True, stop=not last)
                        if last:
                            for s in range(2):
                                nc.tensor.matmul(
                                    pss[s][:, pos + w - 128:pos + w], ident,
                                    maskN, start=False, stop=True)
                        pos += w
                    for s in range(2):
                        nc.scalar.activation(
                            ets[s][:, i * 1024:i * 1024 + tw], pss[s][:, :tw],
                            AF.Exp, scale=SCALE, accum_out=parts[s][:, i:i + 1])
                for s in range(2):
                    if ntile > 1:
                        tsum = small.tile([128, 1], f32, tag="tsum")
                        nc.vector.reduce_sum(out=tsum, in_=parts[s][:, :ntile],
                                             axis=mybir.AxisListType.X)
                        nc.vector.reciprocal(recs[s][:, sqb:sqb + 1], tsum)
                    else:
                        nc.vector.reciprocal(recs[s][:, sqb:sqb + 1],
                                             parts[s][:, 0:1])
                    aw = aw_p.tile([128, S], f32, tag="aw")
                    nc.vector.tensor_scalar_mul(aw[:, :sk_len],
                                                ets[s][:, :sk_len],
                                                recs[s][:, sqb:sqb + 1])
                    nc.sync.dma_start(
                        out=attn_w[2 * pt + s,
                                   sqb * 128:(sqb + 1) * 128, 0:sk_len],
                        in_=aw[:, :sk_len])

            rbs = []
            for s in range(2):
                # broadcast recip over partitions via a DRAM round-trip
                nc.gpsimd.dma_start(
                    out=bass.AP(tensor=rscr.tensor, offset=(2 * pt + s) * S,
                                ap=[[1, 128], [128, NBLK]]),
                    in_=recs[s])
                rb = rb_p.tile([128, S], f32, tag="rb", name=f"rb{s}")
                nc.gpsimd.dma_start(
                    out=rb.rearrange("p (b j) -> p b j", j=128),
                    in_=bass.AP(tensor=rscr.tensor, offset=(2 * pt + s) * S,
                                ap=[[0, 128], [128, NBLK], [1, 128]]))
                rbs.append(rb)

            for s in range(2):
                h = 2 * pt + s
                rb = rbs[s]
                # transposed side: logitsT [sk, sq] -> expT
                eT = expT_p.tile([128, TRI], bf16, tag="eT", name="eT")
                for skc in range(NBLK):
                    sq0 = skc * 128 if causal else 0
                    o_c = TRI_OFF[skc]
                    pos = sq0
                    while pos < S:
                        tw = min(1024, S - pos)
                        ps = lg_ps.tile([128, 1024], f32, tag="lg", name="lgT")
                        ipos = 0
                        while ipos < tw:
                            w = min(512, tw - ipos)
                            diag = causal and (pos == sq0 and ipos == 0)
                            nc.tensor.matmul(
                                ps[:, ipos:ipos + w],
                                ks[s][:, skc * 128:(skc + 1) * 128],
                                qs[s][:, pos + ipos:pos + ipos + w],
                                start=True, stop=not diag)
                            if diag:
                                nc.tensor.matmul(ps[:, 0:128], ident, maskT,
                                                 start=False, stop=True)
                            ipos += w
                        nc.scalar.activation(
                            eT[:, o_c + pos - sq0:o_c + pos - sq0 + tw],
                            ps[:, :tw], AF.Exp, scale=SCALE)
                        pos += tw

                # attn^T [dh, sq] strips: lhsT = v chunk, rhs = expT
                for st in range(4):
                    smax = (st + 1) * 4 if causal else NBLK
                    aps = at_ps.tile([128, 512], f32, tag="at", name="aps")
                    for skc in range(smax):
                        sq0 = skc * 128 if causal else 0
                        cst = st * 512
                        lo = max(cst, sq0)
                        wdt = cst + 512 - lo
                        nc.tensor.matmul(
                            aps[:, lo - cst:512],
                            vA[:, skc, h * 128:(h + 1) * 128],
                            eT[:, TRI_OFF[skc] + lo - sq0:
                               TRI_OFF[skc] + lo - sq0 + wdt],
                            start=(skc == 0), stop=(skc == smax - 1),
                            skip_group_check=True)
                    nc.vector.tensor_mul(
                        attnT[:, h, st * 512:(st + 1) * 512], aps,
                        rb[:, st * 512:(st + 1) * 512])
                    if h == HPC - 1:
                        for sqb in range(st * 4, st * 4 + 4):
                            emit_fc(sqb)

    nc.compile()
    return nc


def _get_program(causal: bool):
    if causal not in _CACHE:
        _CACHE[causal] = _build(causal)
    return _CACHE[causal]


def kernel(q_in, k_in, v_in, mask, Wq_w, Wq_b, Wk_w, Wk_b, Wv_w, Wv_b,
           fc_w, fc_b):
    global last_result
    q_in = np.asarray(q_in, dtype=np.float32)
    k_in = np.asarray(k_in, dtype=np.float32)
    v_in = np.asarray(v_in, dtype=np.float32)
    mask = np.asarray(mask).astype(bool).reshape(S, S)
    Wq_w = np.asarray(Wq_w, dtype=np.float32)
    Wq_b = np.asarray(Wq_b, dtype=np.float32)
    Wk_w = np.asarray(Wk_w, dtype=np.float32)
    Wk_b = np.asarray(Wk_b, dtype=np.float32)
    Wv_w = np.asarray(Wv_w, dtype=np.float32)
    Wv_b = np.asarray(Wv_b, dtype=np.float32)
    fc_w = np.asarray(fc_w, dtype=np.float32)
    fc_b = np.asarray(fc_b, dtype=np.float32)

    causal_ref = np.triu(np.ones((S, S), dtype=bool), k=1)
    if np.array_equal(mask, causal_ref):
        causal = True
    elif not mask.any():
        causal = False
    else:
        raise NotImplementedError("only causal or empty masks supported")

    nc = _get_program(causal)

    qb = [_to_bf16(q_in[b]) for b in range(B)]
    kb = [_to_bf16(k_in[b]) for b in range(B)]
    vb = [_to_bf16(v_in[b]) for b in range(B)]

    in_maps = []
    for core in range(8):
        b, g = divmod(core, 2)
        wq_s = Wq_w[:, g * HPC * DQK:(g + 1) * HPC * DQK]       # [1024, 256]
        wk_s = Wk_w[:, g * HPC * DQK:(g + 1) * HPC * DQK]
        wv_s = Wv_w[:, g * HPC * DH:(g + 1) * HPC * DH]         # [1024, 512]
        wfc_s = fc_w[g * HPC * DH:(g + 1) * HPC * DH, :]        # [512, 1024]
        in_maps.append({
            "q_in": qb[b],
            "k_in": kb[b],
            "v_in": vb[b],
            "wq": _to_bf16(wq_s.reshape(8, 128, 2 * DH).transpose(1, 0, 2)),
            "wk": _to_bf16(wk_s.reshape(8, 128, 2 * DH).transpose(1, 0, 2)),
            "wv": _to_bf16(wv_s.reshape(8, 128, 4 * DH).transpose(1, 0, 2)),
            "wfc": _to_bf16(wfc_s.reshape(HPC, 128, D).transpose(1, 0, 2)),
            "bq": _to_bf16(Wq_b[g * HPC * DQK:(g + 1) * HPC * DQK].reshape(1, -1)),
            "bk": _to_bf16(Wk_b[g * HPC * DQK:(g + 1) * HPC * DQK].reshape(1, -1)),
            "bv": _to_bf16(Wv_b[g * HPC * DH:(g + 1) * HPC * DH].reshape(1, -1)),
            "bfc": _to_bf16(fc_b.reshape(1, D)),
        })

    trace = os.environ.get("KERNEL_TRACE") == "1"
    if trace:
        try:
            import antenv.axon_hooks  # noqa: F401  (wired by test harness)
        except ImportError:
            trace = False
    res = run_bass_kernel_spmd(nc, in_maps, core_ids=list(range(8)), trace=trace)
    last_result = res

    out = np.zeros((B, S, D), dtype=np.float32)
    attn = np.empty((B, H, S, S), dtype=np.float32)
    for core in range(8):
        b, g = divmod(core, 2)
        out[b] += res.results[core]["out_p"]
        attn[b, g * HPC:(g + 1) * HPC] = res.results[core]["attn_w"]
    return out, attn


# revision 20
# speedup vs baseline: 1.0095x; 1.0095x over previous
"""Trainium2 Bass kernel for BaselineAttention (B=4, S=2048, D=1024, H=8).

Sharding: 8 cores = 4 batches x 2 head-groups (4 heads each).
Each core computes, for its (batch, 4 heads):
  q/k/v projections (bf16 matmuls, fp32 accum), causal attention with
  fp32 softmax (exp on ACT with fused row-sums), attention-weight output,
  and its row-parallel slice of the final fc (partials summed on host).

Inputs/weights are cast to bf16 on the host (part of the sharding/layout
step); transposed activation layouts are produced by the DMA xbar
transpose directly from DRAM. Logits are computed in both [sq,sk] and
[sk,sq] orientations on the TensorEngine (cheaper than transposing the
softmax output on-chip); q/k head pairs share the PE array via row
groups (K=64 each).
"""

import math
import os
from contextlib import ExitStack

import ml_dtypes
import numpy as np

import concourse.bass as bass
import concourse.mybir as mybir
import concourse.tile as tile
from concourse import bacc
from concourse.bass_utils import run_bass_kernel_spmd
from concourse.masks import make_causal_mask, make_identity

B, S, D, H = 4, 2048, 1024, 8
DH = D // H            # 128 (v head dim)
DQK = (D // 2) // H    # 64  (q/k head dim)
HPC = H // 2           # 4 heads per core
NBLK = S // 128        # 16
SCALE = 1.0 / math.sqrt(DH)
NEG = -1e9

bf16 = mybir.dt.bfloat16
f32 = mybir.dt.float32
AF = mybir.ActivationFunctionType

_CACHE = {}
last_result = None  # stashed BassKernelResults when KERNEL_TRACE=1


def _to_bf16(x):
    """Round-to-nearest-even float32 -> bfloat16, numpy only."""
    u = np.ascontiguousarray(x, dtype=np.float32).view(np.uint32)
    r = (u >> 16) & np.uint32(1)
    out = ((u + np.uint32(0x7FFF) + r) >> 16).astype(np.uint16)
    return out.view(ml_dtypes.bfloat16)


def _tri_offsets(causal):
    """Column offsets of each sk-chunk's row inside the triangular expT tile."""
    offs, tot = [], 0
    for c in range(NBLK):
        offs.append(tot)
        tot += (S - c * 128) if causal else S
    return offs, tot


def _build(causal: bool):
    nc = bacc.Bacc("TRN2", target_bir_lowering=False, debug=False)

    q_in = nc.dram_tensor("q_in", [S, D], bf16, kind="ExternalInput").ap()
    k_in = nc.dram_tensor("k_in", [S, D], bf16, kind="ExternalInput").ap()
    v_in = nc.dram_tensor("v_in", [S, D], bf16, kind="ExternalInput").ap()
    # weights pre-arranged host-side: [partition, k-chunk, out-cols]
    wq = nc.dram_tensor("wq", [128, 8, 2 * DH], bf16, kind="ExternalInput").ap()
    wk = nc.dram_tensor("wk", [128, 8, 2 * DH], bf16, kind="ExternalInput").ap()
    wv = nc.dram_tensor("wv", [128, 8, 4 * DH], bf16, kind="ExternalInput").ap()
    wfc = nc.dram_tensor("wfc", [128, HPC, D], bf16, kind="ExternalInput").ap()
    # bias rows (added via K=1 rank-1 matmuls into the psum accumulation)
    bq = nc.dram_tensor("bq", [1, 2 * DH], bf16, kind="ExternalInput").ap()
    bk = nc.dram_tensor("bk", [1, 2 * DH], bf16, kind="ExternalInput").ap()
    bv = nc.dram_tensor("bv", [1, 4 * DH], bf16, kind="ExternalInput").ap()
    bfc = nc.dram_tensor("bfc", [1, D], bf16, kind="ExternalInput").ap()

    rscr = nc.dram_tensor("rscr", [HPC, S], f32).ap()
    attn_w = nc.dram_tensor("attn_w", [HPC, S, S], f32, kind="ExternalOutput").ap()
    out_p = nc.dram_tensor("out_p", [S, D], f32, kind="ExternalOutput").ap()

    TRI_OFF, TRI = _tri_offsets(causal)

    with tile.TileContext(nc) as tc, ExitStack() as ctx:
        singles = ctx.enter_context(tc.tile_pool(name="singles", bufs=1))
        ident = singles.tile([128, 128], bf16)
        make_identity(nc, ident)
        if causal:
            maskN = singles.tile([128, 128], bf16)
            make_causal_mask(nc, maskN, mask_val=NEG)  # fill where col > row
            maskT = singles.tile([128, 128], bf16)     # fill where row > col
            nc.gpsimd.memset(maskT, 0.0)
            nc.gpsimd.affine_select(
                out=maskT, in_=maskT, compare_op=mybir.AluOpType.is_ge,
                fill=NEG, base=0, pattern=[[1, 128]], channel_multiplier=-1,
            )
        # weights via SWDGE so the HWDGE xbar stays in transpose mode
        wq_sb = singles.tile([128, 8, 2 * DH], bf16)
        nc.gpsimd.dma_start(out=wq_sb, in_=wq)
        wk_sb = singles.tile([128, 8, 2 * DH], bf16)
        nc.gpsimd.dma_start(out=wk_sb, in_=wk)
        wv_sb = singles.tile([128, 8, 4 * DH], bf16)
        nc.gpsimd.dma_start(out=wv_sb, in_=wv)
        wfc_sb = singles.tile([128, HPC, D], bf16)
        nc.gpsimd.dma_start(out=wfc_sb, in_=wfc)
        bq_sb = singles.tile([1, 2 * DH], bf16)
        nc.gpsimd.dma_start(out=bq_sb, in_=bq)
        bk_sb = singles.tile([1, 2 * DH], bf16)
        nc.gpsimd.dma_start(out=bk_sb, in_=bk)
        bv_sb = singles.tile([1, 4 * DH], bf16)
        nc.gpsimd.dma_start(out=bv_sb, in_=bv)
        bfc_sb = singles.tile([1, D], bf16)
        nc.gpsimd.dma_start(out=bfc_sb, in_=bfc)
        ones1 = singles.tile([1, 512], bf16)
        nc.vector.memset(ones1, 1.0)

        acts = ctx.enter_context(tc.tile_pool(name="acts", bufs=1))
        qT = [acts.tile([128, S], bf16, tag=f"qT{i}", name=f"qT{i}")
              for i in range(2)]
        kT = [acts.tile([128, S], bf16, tag=f"kT{i}", name=f"kT{i}")
              for i in range(2)]
        vA = acts.tile([128, NBLK, 4 * DH], bf16, tag="v")
        attnT = acts.tile([128, HPC, S], bf16, tag="attnT")

        # ---- transpose-load inputs (DMA xbar from DRAM bf16), project ----
        with tc.tile_pool(name="xT", bufs=2) as xTp, \
             tc.tile_pool(name="projps", bufs=2, space="PSUM") as projps:

            def load_xT(x_dram):
                xT_t = xTp.tile([128, 8, S], bf16, tag="xT")
                nc.sync.dma_start(out=xT_t, in_=x_dram, transpose=True)
                return xT_t

            for x_dram, w_sb, b_sb, dstT in ((q_in, wq_sb, bq_sb, qT),
                                             (k_in, wk_sb, bk_sb, kT)):
                xt = load_xT(x_dram)
                for pair in range(2):
                    for st in range(4):
                        ps = projps.tile([128, 512], f32, tag="pps")
                        for c in range(8):
                            nc.tensor.matmul(
                                ps, w_sb[:, c, pair * 128:(pair + 1) * 128],
                                xt[:, c, st * 512:(st + 1) * 512],
                                start=(c == 0), stop=False)
                        # bias: column (x) ones-row outer product
                        nc.tensor.matmul(
                            ps, b_sb[:, pair * 128:(pair + 1) * 128],
                            ones1, start=False, stop=True)
                        nc.vector.tensor_copy(
                            dstT[pair][:, st * 512:(st + 1) * 512], ps)
            xt = load_xT(v_in)
            for b in range(NBLK):
                ps = projps.tile([128, 512], f32, tag="pps")
                for c in range(8):
                    nc.tensor.matmul(ps, xt[:, c, b * 128:(b + 1) * 128],
                                     wv_sb[:, c, :], start=(c == 0), stop=False)
                nc.tensor.matmul(ps, ones1[:, 0:128], bv_sb,
                                 start=False, stop=True)
                nc.vector.tensor_copy(vA[:, b, :], ps)

        # ---- main attention pools ----
        lg_ps = ctx.enter_context(tc.tile_pool(name="lgps", bufs=3, space="PSUM"))
        at_ps = ctx.enter_context(tc.tile_pool(name="atps", bufs=2, space="PSUM"))
        expnat = ctx.enter_context(tc.tile_pool(name="expnat", bufs=4))
        expT_p = ctx.enter_context(tc.tile_pool(name="expT", bufs=1))
        aw_p = ctx.enter_context(tc.tile_pool(name="aw", bufs=3))
        small = ctx.enter_context(tc.tile_pool(name="small", bufs=6))
        rec_p = ctx.enter_context(tc.tile_pool(name="rec", bufs=2))
        rb_p = ctx.enter_context(tc.tile_pool(name="rb", bufs=3))
        out_sb = ctx.enter_context(tc.tile_pool(name="outsb", bufs=2))

        def emit_fc(sqb):
            ot = out_sb.tile([128, D], f32, tag="out", name="ot")
            for st in range(2):
                ps = at_ps.tile([128, 512], f32, tag="at", name="fps")
                for c in range(HPC):
                    nc.tensor.matmul(ps, attnT[:, c, sqb * 128:(sqb + 1) * 128],
                                     wfc_sb[:, c, st * 512:(st + 1) * 512],
                                     start=(c == 0), stop=False)
                nc.tensor.matmul(ps, ones1[:, 0:128],
                                 bfc_sb[:, st * 512:(st + 1) * 512],
                                 start=False, stop=True)
                nc.vector.tensor_copy(ot[:, st * 512:(st + 1) * 512], ps)
            nc.gpsimd.dma_start(out=out_p[sqb * 128:(sqb + 1) * 128, :], in_=ot)

        for pt in range(2):
            qs = [qT[pt][0:64, :], qT[pt][64:128, :]]
            ks = [kT[pt][0:64, :], kT[pt][64:128, :]]
            recs = [rec_p.tile([128, NBLK], f32, tag="rec", name=f"rec{s}")
                    for s in range(2)]

            # natural side, head pair packed into PE row groups
            for sqb in range(NBLK):
                sk_len = (sqb + 1) * 128 if causal else S
                ntile = (sk_len + 1023) // 1024
                ets = [expnat.tile([128, S], bf16, tag="expnat", name=f"et{s}")
                       for s in range(2)]
                parts = [small.tile([128, 2], f32, tag="parts", name=f"pt{s}")
                         for s in range(2)]
                for i in range(ntile):
                    tw = min(1024, sk_len - i * 1024)
                    pss = [lg_ps.tile([128, 1024], f32, tag="lg", name=f"lg{s}")
                           for s in range(2)]
                    pos = 0
                    while pos < tw:
                        w = min(512, tw - pos)
                        col = i * 1024 + pos
                        last = causal and (col + w == sk_len)
                        for s in range(2):
                            nc.tensor.matmul(
                                pss[s][:, pos:pos + w],
                                qs[s][:, sqb * 128:(sqb + 1) * 128],
                                ks[s][:, col:col + w],
                                start=True, stop=not last)
                        if last:
                            for s in range(2):
                                nc.tensor.matmul(
                                    pss[s][:, pos + w - 128:pos + w], ident,
                                    maskN, start=False, stop=True)
                        pos += w
                    for s in range(2):
                        nc.scalar.activation(
                            ets[s][:, i * 1024:i * 1024 + tw], pss[s][:, :tw],
                            AF.Exp, scale=SCALE, accum_out=parts[s][:, i:i + 1])
                for s in range(2):
                    if ntile > 1:
                        tsum = small.tile([128, 1], f32, tag="tsum")
                        nc.vector.reduce_sum(out=tsum, in_=parts[s][:, :ntile],
                                             axis=mybir.AxisListType.X)
                        nc.vector.reciprocal(recs[s][:, sqb:sqb + 1], tsum)
                    else:
                        nc.vector.reciprocal(recs[s][:, sqb:sqb + 1],
                                             parts[s][:, 0:1])
                    aw = aw_p.tile([128, S], f32, tag="aw")
                    nc.vector.tensor_scalar_mul(aw[:, :sk_len],
                                                ets[s][:, :sk_len],
                                                recs[s][:, sqb:sqb + 1])
                    nc.sync.dma_start(
                        out=attn_w[2 * pt + s,
                                   sqb * 128:(sqb + 1) * 128, 0:sk_len],
                        in_=aw[:, :sk_len])

            rbs = []
            for s in range(2):
                # broadcast recip over partitions via a DRAM round-trip
                nc.gpsimd.dma_start(
                    out=bass.AP(tensor=rscr.tensor, offset=(2 * pt + s) * S,
                                ap=[[1, 128], [128, NBLK]]),
                    in_=recs[s])
                rb = rb_p.tile([128, S], f32, tag="rb", name=f"rb{s}")
                nc.gpsimd.dma_start(
                    out=rb.rearrange("p (b j) -> p b j", j=128),
                    in_=bass.AP(tensor=rscr.tensor, offset=(2 * pt + s) * S,
                                ap=[[0, 128], [128, NBLK], [1, 128]]))
                rbs.append(rb)

            for s in range(2):
                h = 2 * pt + s
                rb = rbs[s]
                # transposed side: logitsT [sk, sq] -> expT
                eT = expT_p.tile([128, TRI], bf16, tag="eT", name="eT")
                for skc in range(NBLK):
                    sq0 = skc * 128 if causal else 0
                    o_c = TRI_OFF[skc]
                    pos = sq0
                    while pos < S:
                        tw = min(1024, S - pos)
                        ps = lg_ps.tile([128, 1024], f32, tag="lg", name="lgT")
                        ipos = 0
                        while ipos < tw:
                            w = min(512, tw - ipos)
                            diag = causal and (pos == sq0 and ipos == 0)
                            nc.tensor.matmul(
                                ps[:, ipos:ipos + w],
                                ks[s][:, skc * 128:(skc + 1) * 128],
                                qs[s][:, pos + ipos:pos + ipos + w],
                                start=True, stop=not diag)
                            if diag:
                                nc.tensor.matmul(ps[:, 0:128], ident, maskT,
                                                 start=False, stop=True)
                            ipos += w
                        nc.scalar.activation(
                            eT[:, o_c + pos - sq0:o_c + pos - sq0 + tw],
                            ps[:, :tw], AF.Exp, scale=SCALE)
                        pos += tw

                # attn^T [dh, sq] strips: lhsT = v chunk, rhs = expT
                for st in range(4):
                    smax = (st + 1) * 4 if causal else NBLK
                    aps = at_ps.tile([128, 512], f32, tag="at", name="aps")
                    for skc in range(smax):
                        sq0 = skc * 128 if causal else 0
                        cst = st * 512
                        lo = max(cst, sq0)
                        wdt = cst + 512 - lo
                        nc.tensor.matmul(
                            aps[:, lo - cst:512],
                            vA[:, skc, h * 128:(h + 1) * 128],
                            eT[:, TRI_OFF[skc] + lo - sq0:
                               TRI_OFF[skc] + lo - sq0 + wdt],
                            start=(skc == 0), stop=(skc == smax - 1),
                            skip_group_check=True)
                    nc.vector.tensor_mul(
                        attnT[:, h, st * 512:(st + 1) * 512], aps,
                        rb[:, st * 512:(st + 1) * 512])
                    if h == HPC - 1:
                        for sqb in range(st * 4, st * 4 + 4):
                            emit_fc(sqb)

    nc.compile()
    return nc


def _get_program(causal: bool):
    if causal not in _CACHE:
        _CACHE[causal] = _build(causal)
    return _CACHE[causal]


def kernel(q_in, k_in, v_in, mask, Wq_w, Wq_b, Wk_w, Wk_b, Wv_w, Wv_b,
           fc_w, fc_b):
    global last_result
    q_in = np.asarray(q_in, dtype=np.float32)
    k_in = np.asarray(k_in, dtype=np.float32)
    v_in = np.asarray(v_in, dtype=np.float32)
    mask = np.asarray(mask).astype(bool).reshape(S, S)
    Wq_w = np.asarray(Wq_w, dtype=np.float32)
    Wq_b = np.asarray(Wq_b, dtype=np.float32)
    Wk_w = np.asarray(Wk_w, dtype=np.float32)
    Wk_b = np.asarray(Wk_b, dtype=np.float32)
    Wv_w = np.asarray(Wv_w, dtype=np.float32)
    Wv_b = np.asarray(Wv_b, dtype=np.float32)
    fc_w = np.asarray(fc_w, dtype=np.float32)
    fc_b = np.asarray(fc_b, dtype=np.float32)

    causal_ref = np.triu(np.ones((S, S), dtype=bool), k=1)
    if np.array_equal(mask, causal_ref):
        causal = True
    elif not mask.any():
        causal = False
    else:
        raise NotImplementedError("only causal or empty masks supported")

    nc = _get_program(causal)

    qb = [_to_bf16(q_in[b]) for b in range(B)]
    kb = [_to_bf16(k_in[b]) for b in range(B)]
    vb = [_to_bf16(v_in[b]) for b in range(B)]

    in_maps = []
    for core in range(8):
        b, g = divmod(core, 2)
        wq_s = Wq_w[:, g * HPC * DQK:(g + 1) * HPC * DQK]       # [1024, 256]
        wk_s = Wk_w[:, g * HPC * DQK:(g + 1) * HPC * DQK]
        wv_s = Wv_w[:, g * HPC * DH:(g + 1) * HPC * DH]         # [1024, 512]
        wfc_s = fc_w[g * HPC * DH:(g + 1) * HPC * DH, :]        # [512, 1024]
        in_maps.append({
            "q_in": qb[b],
            "k_in": kb[b],
            "v_in": vb[b],
            "wq": _to_bf16(wq_s.reshape(8, 128, 2 * DH).transpose(1, 0, 2)),
            "wk": _to_bf16(wk_s.reshape(8, 128, 2 * DH).transpose(1, 0, 2)),
            "wv": _to_bf16(wv_s.reshape(8, 128, 4 * DH).transpose(1, 0, 2)),
            "wfc": _to_bf16(wfc_s.reshape(HPC, 128, D).transpose(1, 0, 2)),
            "bq": _to_bf16(Wq_b[g * HPC * DQK:(g + 1) * HPC * DQK].reshape(1, -1)),
            "bk": _to_bf16(Wk_b[g * HPC * DQK:(g + 1) * HPC * DQK].reshape(1, -1)),
            "bv": _to_bf16(Wv_b[g * HPC * DH:(g + 1) * HPC * DH].reshape(1, -1)),
            "bfc": _to_bf16(fc_b.reshape(1, D)),
        })

    trace = os.environ.get("KERNEL_TRACE") == "1"
    if trace:
        try:
            import antenv.axon_hooks  # noqa: F401  (wired by test harness)
        except ImportError:
            trace = False
    res = run_bass_kernel_spmd(nc, in_maps, core_ids=list(range(8)), trace=trace)
    last_result = res

    out = np.zeros((B, S, D), dtype=np.float32)
    attn = np.empty((B, H, S, S), dtype=np.float32)
    for core in range(8):
        b, g = divmod(core, 2)
        out[b] += res.results[core]["out_p"]
        attn[b, g * HPC:(g + 1) * HPC] = res.results[core]["attn_w"]
    return out, attn


# revision 21
# speedup vs baseline: 1.0373x; 1.0276x over previous
"""Trainium2 Bass kernel for BaselineAttention (B=4, S=2048, D=1024, H=8).

Sharding: 8 cores = 4 batches x 2 head-groups (4 heads each).
Each core computes, for its (batch, 4 heads):
  q/k/v projections (bf16 matmuls, fp32 accum), causal attention with
  fp32 softmax (exp on ACT with fused row-sums), attention-weight output,
  and its row-parallel slice of the final fc (partials summed on host).

Inputs/weights are cast to bf16 on the host (part of the sharding/layout
step); transposed activation layouts are produced by the DMA xbar
transpose directly from DRAM. Logits are computed in both [sq,sk] and
[sk,sq] orientations on the TensorEngine (cheaper than transposing the
softmax output on-chip); q/k head pairs share the PE array via row
groups (K=64 each).
"""

import math
import os
from contextlib import ExitStack

import ml_dtypes
import numpy as np

import concourse.bass as bass
import concourse.mybir as mybir
import concourse.tile as tile
from concourse import bacc
from concourse.bass_utils import run_bass_kernel_spmd
from concourse.masks import make_causal_mask, make_identity

B, S, D, H = 4, 2048, 1024, 8
DH = D // H            # 128 (v head dim)
DQK = (D // 2) // H    # 64  (q/k head dim)
HPC = H // 2           # 4 heads per core
NBLK = S // 128        # 16
SCALE = 1.0 / math.sqrt(DH)
NEG = -1e9

bf16 = mybir.dt.bfloat16
f32 = mybir.dt.float32
AF = mybir.ActivationFunctionType

_CACHE = {}
last_result = None  # stashed BassKernelResults when KERNEL_TRACE=1


def _to_bf16(x):
    """Round-to-nearest-even float32 -> bfloat16, numpy only."""
    u = np.ascontiguousarray(x, dtype=np.float32).view(np.uint32)
    r = (u >> 16) & np.uint32(1)
    out = ((u + np.uint32(0x7FFF) + r) >> 16).astype(np.uint16)
    return out.view(ml_dtypes.bfloat16)


def _tri_offsets(causal):
    """Column offsets of each sk-chunk's row inside the triangular expT tile."""
    offs, tot = [], 0
    for c in range(NBLK):
        offs.append(tot)
        tot += (S - c * 128) if causal else S
    return offs, tot


def _build(causal: bool):
    nc = bacc.Bacc("TRN2", target_bir_lowering=False, debug=False)

    q_in = nc.dram_tensor("q_in", [S, D], bf16, kind="ExternalInput").ap()
    k_in = nc.dram_tensor("k_in", [S, D], bf16, kind="ExternalInput").ap()
    v_in = nc.dram_tensor("v_in", [S, D], bf16, kind="ExternalInput").ap()
    # weights pre-arranged host-side: [partition, k-chunk, out-cols]
    wq = nc.dram_tensor("wq", [128, 8, 2 * DH], bf16, kind="ExternalInput").ap()
    wk = nc.dram_tensor("wk", [128, 8, 2 * DH], bf16, kind="ExternalInput").ap()
    wv = nc.dram_tensor("wv", [128, 8, 4 * DH], bf16, kind="ExternalInput").ap()
    wfc = nc.dram_tensor("wfc", [128, HPC, D], bf16, kind="ExternalInput").ap()
    # bias rows (added via K=1 rank-1 matmuls into the psum accumulation)
    bq = nc.dram_tensor("bq", [1, 2 * DH], bf16, kind="ExternalInput").ap()
    bk = nc.dram_tensor("bk", [1, 2 * DH], bf16, kind="ExternalInput").ap()
    bv = nc.dram_tensor("bv", [1, 4 * DH], bf16, kind="ExternalInput").ap()
    bfc = nc.dram_tensor("bfc", [1, D], bf16, kind="ExternalInput").ap()

    rscr = nc.dram_tensor("rscr", [HPC, S], f32).ap()
    attn_w = nc.dram_tensor("attn_w", [HPC, S, S], f32, kind="ExternalOutput").ap()
    out_p = nc.dram_tensor("out_p", [S, D], f32, kind="ExternalOutput").ap()

    TRI_OFF, TRI = _tri_offsets(causal)

    with tile.TileContext(nc) as tc, ExitStack() as ctx:
        singles = ctx.enter_context(tc.tile_pool(name="singles", bufs=1))
        ident = singles.tile([128, 128], bf16)
        make_identity(nc, ident)
        if causal:
            maskN = singles.tile([128, 128], bf16)
            make_causal_mask(nc, maskN, mask_val=NEG)  # fill where col > row
            maskT = singles.tile([128, 128], bf16)     # fill where row > col
            nc.gpsimd.memset(maskT, 0.0)
            nc.gpsimd.affine_select(
                out=maskT, in_=maskT, compare_op=mybir.AluOpType.is_ge,
                fill=NEG, base=0, pattern=[[1, 128]], channel_multiplier=-1,
            )
        # weights via SWDGE so the HWDGE xbar stays in transpose mode
        wq_sb = singles.tile([128, 8, 2 * DH], bf16)
        nc.gpsimd.dma_start(out=wq_sb, in_=wq)
        wk_sb = singles.tile([128, 8, 2 * DH], bf16)
        nc.gpsimd.dma_start(out=wk_sb, in_=wk)
        wv_sb = singles.tile([128, 8, 4 * DH], bf16)
        nc.gpsimd.dma_start(out=wv_sb, in_=wv)
        wfc_sb = singles.tile([128, HPC, D], bf16)
        nc.gpsimd.dma_start(out=wfc_sb, in_=wfc)
        bq_sb = singles.tile([1, 2 * DH], bf16)
        nc.gpsimd.dma_start(out=bq_sb, in_=bq)
        bk_sb = singles.tile([1, 2 * DH], bf16)
        nc.gpsimd.dma_start(out=bk_sb, in_=bk)
        bv_sb = singles.tile([1, 4 * DH], bf16)
        nc.gpsimd.dma_start(out=bv_sb, in_=bv)
        bfc_sb = singles.tile([1, D], bf16)
        nc.gpsimd.dma_start(out=bfc_sb, in_=bfc)
        ones1 = singles.tile([1, 512], bf16)
        nc.vector.memset(ones1, 1.0)

        acts = ctx.enter_context(tc.tile_pool(name="acts", bufs=1))
        qT = [acts.tile([128, S], bf16, tag=f"qT{i}", name=f"qT{i}")
              for i in range(2)]
        kT = [acts.tile([128, S], bf16, tag=f"kT{i}", name=f"kT{i}")
              for i in range(2)]
        vA = acts.tile([128, NBLK, 4 * DH], bf16, tag="v")
        attnT = acts.tile([128, HPC, S], bf16, tag="attnT")

        # ---- transpose-load inputs (DMA xbar from DRAM bf16), project ----
        with tc.tile_pool(name="xT", bufs=2) as xTp, \
             tc.tile_pool(name="projps", bufs=2, space="PSUM") as projps:

            def load_xT(x_dram):
                xT_t = xTp.tile([128, 8, S], bf16, tag="xT")
                nc.sync.dma_start(out=xT_t[:, 0:4, :], in_=x_dram[:, 0:512],
                                  transpose=True)
                nc.sync.dma_start(out=xT_t[:, 4:8, :], in_=x_dram[:, 512:1024],
                                  transpose=True)
                return xT_t

            for x_dram, w_sb, b_sb, dstT in ((q_in, wq_sb, bq_sb, qT),
                                             (k_in, wk_sb, bk_sb, kT)):
                xt = load_xT(x_dram)
                for pair in range(2):
                    for st in range(4):
                        ps = projps.tile([128, 512], f32, tag="pps")
                        for c in range(8):
                            nc.tensor.matmul(
                                ps, w_sb[:, c, pair * 128:(pair + 1) * 128],
                                xt[:, c, st * 512:(st + 1) * 512],
                                start=(c == 0), stop=False)
                        # bias: column (x) ones-row outer product
                        nc.tensor.matmul(
                            ps, b_sb[:, pair * 128:(pair + 1) * 128],
                            ones1, start=False, stop=True)
                        nc.vector.tensor_copy(
                            dstT[pair][:, st * 512:(st + 1) * 512], ps)
            xt = load_xT(v_in)
            for b in range(NBLK):
                ps = projps.tile([128, 512], f32, tag="pps")
                for c in range(8):
                    nc.tensor.matmul(ps, xt[:, c, b * 128:(b + 1) * 128],
                                     wv_sb[:, c, :], start=(c == 0), stop=False)
                nc.tensor.matmul(ps, ones1[:, 0:128], bv_sb,
                                 start=False, stop=True)
                nc.vector.tensor_copy(vA[:, b, :], ps)

        # ---- main attention pools ----
        lg_ps = ctx.enter_context(tc.tile_pool(name="lgps", bufs=3, space="PSUM"))
        at_ps = ctx.enter_context(tc.tile_pool(name="atps", bufs=2, space="PSUM"))
        expnat = ctx.enter_context(tc.tile_pool(name="expnat", bufs=4))
        expT_p = ctx.enter_context(tc.tile_pool(name="expT", bufs=1))
        aw_p = ctx.enter_context(tc.tile_pool(name="aw", bufs=3))
        small = ctx.enter_context(tc.tile_pool(name="small", bufs=6))
        rec_p = ctx.enter_context(tc.tile_pool(name="rec", bufs=2))
        rb_p = ctx.enter_context(tc.tile_pool(name="rb", bufs=3))
        out_sb = ctx.enter_context(tc.tile_pool(name="outsb", bufs=2))

        def emit_fc(sqb):
            ot = out_sb.tile([128, D], f32, tag="out", name="ot")
            for st in range(2):
                ps = at_ps.tile([128, 512], f32, tag="at", name="fps")
                for c in range(HPC):
                    nc.tensor.matmul(ps, attnT[:, c, sqb * 128:(sqb + 1) * 128],
                                     wfc_sb[:, c, st * 512:(st + 1) * 512],
                                     start=(c == 0), stop=False)
                nc.tensor.matmul(ps, ones1[:, 0:128],
                                 bfc_sb[:, st * 512:(st + 1) * 512],
                                 start=False, stop=True)
                nc.vector.tensor_copy(ot[:, st * 512:(st + 1) * 512], ps)
            nc.gpsimd.dma_start(out=out_p[sqb * 128:(sqb + 1) * 128, :], in_=ot)

        for pt in range(2):
            qs = [qT[pt][0:64, :], qT[pt][64:128, :]]
            ks = [kT[pt][0:64, :], kT[pt][64:128, :]]
            recs = [rec_p.tile([128, NBLK], f32, tag="rec", name=f"rec{s}")
                    for s in range(2)]

            # natural side, head pair packed into PE row groups
            for sqb in range(NBLK):
                sk_len = (sqb + 1) * 128 if causal else S
                ntile = (sk_len + 1023) // 1024
                ets = [expnat.tile([128, S], bf16, tag="expnat", name=f"et{s}")
                       for s in range(2)]
                parts = [small.tile([128, 2], f32, tag="parts", name=f"pt{s}")
                         for s in range(2)]
                for i in range(ntile):
                    tw = min(1024, sk_len - i * 1024)
                    pss = [lg_ps.tile([128, 1024], f32, tag="lg", name=f"lg{s}")
                           for s in range(2)]
                    pos = 0
                    while pos < tw:
                        w = min(512, tw - pos)
                        col = i * 1024 + pos
                        last = causal and (col + w == sk_len)
                        for s in range(2):
                            nc.tensor.matmul(
                                pss[s][:, pos:pos + w],
                                qs[s][:, sqb * 128:(sqb + 1) * 128],
                                ks[s][:, col:col + w],
                                start=True, stop=not last)
                        if last:
                            for s in range(2):
                                nc.tensor.matmul(
                                    pss[s][:, pos + w - 128:pos + w], ident,
                                    maskN, start=False, stop=True)
                        pos += w
                    for s in range(2):
                        nc.scalar.activation(
                            ets[s][:, i * 1024:i * 1024 + tw], pss[s][:, :tw],
                            AF.Exp, scale=SCALE, accum_out=parts[s][:, i:i + 1])
                for s in range(2):
                    if ntile > 1:
                        tsum = small.tile([128, 1], f32, tag="tsum")
                        nc.vector.reduce_sum(out=tsum, in_=parts[s][:, :ntile],
                                             axis=mybir.AxisListType.X)
                        nc.vector.reciprocal(recs[s][:, sqb:sqb + 1], tsum)
                    else:
                        nc.vector.reciprocal(recs[s][:, sqb:sqb + 1],
                                             parts[s][:, 0:1])
                    aw = aw_p.tile([128, S], f32, tag="aw")
                    nc.vector.tensor_scalar_mul(aw[:, :sk_len],
                                                ets[s][:, :sk_len],
                                                recs[s][:, sqb:sqb + 1])
                    nc.sync.dma_start(
                        out=attn_w[2 * pt + s,
                                   sqb * 128:(sqb + 1) * 128, 0:sk_len],
                        in_=aw[:, :sk_len])

            rbs = []
            for s in range(2):
                # broadcast recip over partitions via a DRAM round-trip
                nc.gpsimd.dma_start(
                    out=bass.AP(tensor=rscr.tensor, offset=(2 * pt + s) * S,
                                ap=[[1, 128], [128, NBLK]]),
                    in_=recs[s])
                rb = rb_p.tile([128, S], f32, tag="rb", name=f"rb{s}")
                nc.gpsimd.dma_start(
                    out=rb.rearrange("p (b j) -> p b j", j=128),
                    in_=bass.AP(tensor=rscr.tensor, offset=(2 * pt + s) * S,
                                ap=[[0, 128], [128, NBLK], [1, 128]]))
                rbs.append(rb)

            for s in range(2):
                h = 2 * pt + s
                rb = rbs[s]
                # transposed side: logitsT [sk, sq] -> expT
                eT = expT_p.tile([128, TRI], bf16, tag="eT", name="eT")
                for skc in range(NBLK):
                    sq0 = skc * 128 if causal else 0
                    o_c = TRI_OFF[skc]
                    pos = sq0
                    while pos < S:
                        tw = min(1024, S - pos)
                        ps = lg_ps.tile([128, 1024], f32, tag="lg", name="lgT")
                        ipos = 0
                        while ipos < tw:
                            w = min(512, tw - ipos)
                            diag = causal and (pos == sq0 and ipos == 0)
                            nc.tensor.matmul(
                                ps[:, ipos:ipos + w],
                                ks[s][:, skc * 128:(skc + 1) * 128],
                                qs[s][:, pos + ipos:pos + ipos + w],
                                start=True, stop=not diag)
                            if diag:
                                nc.tensor.matmul(ps[:, 0:128], ident, maskT,
                                                 start=False, stop=True)
                            ipos += w
                        nc.scalar.activation(
                            eT[:, o_c + pos - sq0:o_c + pos - sq0 + tw],
                            ps[:, :tw], AF.Exp, scale=SCALE)
                        pos += tw

                # attn^T [dh, sq] strips: lhsT = v chunk, rhs = expT
                for st in range(4):
                    smax = (st + 1) * 4 if causal else NBLK
                    aps = at_ps.tile([128, 512], f32, tag="at", name="aps")
                    for skc in range(smax):
                        sq0 = skc * 128 if causal else 0
                        cst = st * 512
                        lo = max(cst, sq0)
                        wdt = cst + 512 - lo
                        nc.tensor.matmul(
                            aps[:, lo - cst:512],
                            vA[:, skc, h * 128:(h + 1) * 128],
                            eT[:, TRI_OFF[skc] + lo - sq0:
                               TRI_OFF[skc] + lo - sq0 + wdt],
                            start=(skc == 0), stop=(skc == smax - 1),
                            skip_group_check=True)
                    nc.vector.tensor_mul(
                        attnT[:, h, st * 512:(st + 1) * 512], aps,
                        rb[:, st * 512:(st + 1) * 512])
                    if h == HPC - 1:
                        for sqb in range(st * 4, st * 4 + 4):
                            emit_fc(sqb)

    nc.compile()
    return nc


def _get_program(causal: bool):
    if causal not in _CACHE:
        _CACHE[causal] = _build(causal)
    return _CACHE[causal]


def kernel(q_in, k_in, v_in, mask, Wq_w, Wq_b, Wk_w, Wk_b, Wv_w, Wv_b,
           fc_w, fc_b):
    global last_result
    q_in = np.asarray(q_in, dtype=np.float32)
    k_in = np.asarray(k_in, dtype=np.float32)
    v_in = np.asarray(v_in, dtype=np.float32)
    mask = np.asarray(mask).astype(bool).reshape(S, S)
    Wq_w = np.asarray(Wq_w, dtype=np.float32)
    Wq_b = np.asarray(Wq_b, dtype=np.float32)
    Wk_w = np.asarray(Wk_w, dtype=np.float32)
    Wk_b = np.asarray(Wk_b, dtype=np.float32)
    Wv_w = np.asarray(Wv_w, dtype=np.float32)
    Wv_b = np.asarray(Wv_b, dtype=np.float32)
    fc_w = np.asarray(fc_w, dtype=np.float32)
    fc_b = np.asarray(fc_b, dtype=np.float32)

    causal_ref = np.triu(np.ones((S, S), dtype=bool), k=1)
    if np.array_equal(mask, causal_ref):
        causal = True
    elif not mask.any():
        causal = False
    else:
        raise NotImplementedError("only causal or empty masks supported")

    nc = _get_program(causal)

    qb = [_to_bf16(q_in[b]) for b in range(B)]
    kb = [_to_bf16(k_in[b]) for b in range(B)]
    vb = [_to_bf16(v_in[b]) for b in range(B)]

    in_maps = []
    for core in range(8):
        b, g = divmod(core, 2)
        wq_s = Wq_w[:, g * HPC * DQK:(g + 1) * HPC * DQK]       # [1024, 256]
        wk_s = Wk_w[:, g * HPC * DQK:(g + 1) * HPC * DQK]
        wv_s = Wv_w[:, g * HPC * DH:(g + 1) * HPC * DH]         # [1024, 512]
        wfc_s = fc_w[g * HPC * DH:(g + 1) * HPC * DH, :]        # [512, 1024]
        in_maps.append({
            "q_in": qb[b],
            "k_in": kb[b],
            "v_in": vb[b],
            "wq": _to_bf16(wq_s.reshape(8, 128, 2 * DH).transpose(1, 0, 2)),
            "wk": _to_bf16(wk_s.reshape(8, 128, 2 * DH).transpose(1, 0, 2)),
            "wv": _to_bf16(wv_s.reshape(8, 128, 4 * DH).transpose(1, 0, 2)),
            "wfc": _to_bf16(wfc_s.reshape(HPC, 128, D).transpose(1, 0, 2)),
            "bq": _to_bf16(Wq_b[g * HPC * DQK:(g + 1) * HPC * DQK].reshape(1, -1)),
            "bk": _to_bf16(Wk_b[g * HPC * DQK:(g + 1) * HPC * DQK].reshape(1, -1)),
            "bv": _to_bf16(Wv_b[g * HPC * DH:(g + 1) * HPC * DH].reshape(1, -1)),
            "bfc": _to_bf16(fc_b.reshape(1, D)),
        })

    trace = os.environ.get("KERNEL_TRACE") == "1"
    if trace:
        try:
            import antenv.axon_hooks  # noqa: F401  (wired by test harness)
        except ImportError:
            trace = False
    res = run_bass_kernel_spmd(nc, in_maps, core_ids=list(range(8)), trace=trace)
    last_result = res

    out = np.zeros((B, S, D), dtype=np.float32)
    attn = np.empty((B, H, S, S), dtype=np.float32)
    for core in range(8):
        b, g = divmod(core, 2)
        out[b] += res.results[core]["out_p"]
        attn[b, g * HPC:(g + 1) * HPC] = res.results[core]["attn_w"]
    return out, attn


# revision 22
# speedup vs baseline: 1.0441x; 1.0066x over previous
"""Trainium2 Bass kernel for BaselineAttention (B=4, S=2048, D=1024, H=8).

Sharding: 8 cores = 4 batches x 2 head-groups (4 heads each).
Each core computes, for its (batch, 4 heads):
  q/k/v projections (bf16 matmuls, fp32 accum), causal attention with
  fp32 softmax (exp on ACT with fused row-sums), attention-weight output,
  and its row-parallel slice of the final fc (partials summed on host).

Inputs/weights are cast to bf16 on the host (part of the sharding/layout
step); transposed activation layouts are produced by the DMA xbar
transpose directly from DRAM. Logits are computed in both [sq,sk] and
[sk,sq] orientations on the TensorEngine (cheaper than transposing the
softmax output on-chip); q/k head pairs share the PE array via row
groups (K=64 each).
"""

import math
import os
from contextlib import ExitStack

import ml_dtypes
import numpy as np

import concourse.bass as bass
import concourse.mybir as mybir
import concourse.tile as tile
from concourse import bacc
from concourse.bass_utils import run_bass_kernel_spmd
from concourse.masks import make_causal_mask, make_identity

B, S, D, H = 4, 2048, 1024, 8
DH = D // H            # 128 (v head dim)
DQK = (D // 2) // H    # 64  (q/k head dim)
HPC = H // 2           # 4 heads per core
NBLK = S // 128        # 16
SCALE = 1.0 / math.sqrt(DH)
NEG = -1e9

bf16 = mybir.dt.bfloat16
f32 = mybir.dt.float32
AF = mybir.ActivationFunctionType

_CACHE = {}
last_result = None  # stashed BassKernelResults when KERNEL_TRACE=1


def _to_bf16(x):
    """Round-to-nearest-even float32 -> bfloat16, numpy only."""
    u = np.ascontiguousarray(x, dtype=np.float32).view(np.uint32)
    r = (u >> 16) & np.uint32(1)
    out = ((u + np.uint32(0x7FFF) + r) >> 16).astype(np.uint16)
    return out.view(ml_dtypes.bfloat16)


def _tri_offsets(causal):
    """Column offsets of each sk-chunk's row inside the triangular expT tile."""
    offs, tot = [], 0
    for c in range(NBLK):
        offs.append(tot)
        tot += (S - c * 128) if causal else S
    return offs, tot


def _build(causal: bool):
    nc = bacc.Bacc("TRN2", target_bir_lowering=False, debug=False)

    q_in = nc.dram_tensor("q_in", [S, D], bf16, kind="ExternalInput").ap()
    k_in = nc.dram_tensor("k_in", [S, D], bf16, kind="ExternalInput").ap()
    v_in = nc.dram_tensor("v_in", [S, D], bf16, kind="ExternalInput").ap()
    # weights pre-arranged host-side: [partition, k-chunk, out-cols]
    wq = nc.dram_tensor("wq", [128, 8, 2 * DH], bf16, kind="ExternalInput").ap()
    wk = nc.dram_tensor("wk", [128, 8, 2 * DH], bf16, kind="ExternalInput").ap()
    wv = nc.dram_tensor("wv", [128, 8, 4 * DH], bf16, kind="ExternalInput").ap()
    wfc = nc.dram_tensor("wfc", [128, HPC, D], bf16, kind="ExternalInput").ap()
    # bias rows (added via K=1 rank-1 matmuls into the psum accumulation)
    bq = nc.dram_tensor("bq", [1, 2 * DH], bf16, kind="ExternalInput").ap()
    bk = nc.dram_tensor("bk", [1, 2 * DH], bf16, kind="ExternalInput").ap()
    bv = nc.dram_tensor("bv", [1, 4 * DH], bf16, kind="ExternalInput").ap()
    bfc = nc.dram_tensor("bfc", [1, D], bf16, kind="ExternalInput").ap()

    rscr = nc.dram_tensor("rscr", [HPC, S], f32).ap()
    attn_w = nc.dram_tensor("attn_w", [HPC, S, S], f32, kind="ExternalOutput").ap()
    out_p = nc.dram_tensor("out_p", [S, D], f32, kind="ExternalOutput").ap()

    TRI_OFF, TRI = _tri_offsets(causal)

    with tile.TileContext(nc) as tc, ExitStack() as ctx:
        singles = ctx.enter_context(tc.tile_pool(name="singles", bufs=1))
        ident = singles.tile([128, 128], bf16)
        make_identity(nc, ident)
        if causal:
            maskN = singles.tile([128, 128], bf16)
            make_causal_mask(nc, maskN, mask_val=NEG)  # fill where col > row
            maskT = singles.tile([128, 128], bf16)     # fill where row > col
            nc.gpsimd.memset(maskT, 0.0)
            nc.gpsimd.affine_select(
                out=maskT, in_=maskT, compare_op=mybir.AluOpType.is_ge,
                fill=NEG, base=0, pattern=[[1, 128]], channel_multiplier=-1,
            )
        # weights via SWDGE so the HWDGE xbar stays in transpose mode
        wq_sb = singles.tile([128, 8, 2 * DH], bf16)
        nc.gpsimd.dma_start(out=wq_sb, in_=wq)
        wk_sb = singles.tile([128, 8, 2 * DH], bf16)
        nc.gpsimd.dma_start(out=wk_sb, in_=wk)
        wv_sb = singles.tile([128, 8, 4 * DH], bf16)
        nc.gpsimd.dma_start(out=wv_sb, in_=wv)
        wfc_sb = singles.tile([128, HPC, D], bf16)
        nc.gpsimd.dma_start(out=wfc_sb, in_=wfc)
        bq_sb = singles.tile([1, 2 * DH], bf16)
        nc.gpsimd.dma_start(out=bq_sb, in_=bq)
        bk_sb = singles.tile([1, 2 * DH], bf16)
        nc.gpsimd.dma_start(out=bk_sb, in_=bk)
        bv_sb = singles.tile([1, 4 * DH], bf16)
        nc.gpsimd.dma_start(out=bv_sb, in_=bv)
        bfc_sb = singles.tile([1, D], bf16)
        nc.gpsimd.dma_start(out=bfc_sb, in_=bfc)
        ones1 = singles.tile([1, 512], bf16)
        nc.vector.memset(ones1, 1.0)

        acts = ctx.enter_context(tc.tile_pool(name="acts", bufs=1))
        qT = [acts.tile([128, S], bf16, tag=f"qT{i}", name=f"qT{i}")
              for i in range(2)]
        kT = [acts.tile([128, S], bf16, tag=f"kT{i}", name=f"kT{i}")
              for i in range(2)]
        vA = acts.tile([128, NBLK, 4 * DH], bf16, tag="v")
        attnT = acts.tile([128, HPC, S], bf16, tag="attnT")

        # ---- transpose-load inputs (DMA xbar from DRAM bf16), project ----
        with tc.tile_pool(name="xT", bufs=2) as xTp, \
             tc.tile_pool(name="projps", bufs=2, space="PSUM") as projps:

            def load_xT(x_dram):
                xT_t = xTp.tile([128, 8, S], bf16, tag="xT")
                nc.sync.dma_start(out=xT_t[:, 0:4, :], in_=x_dram[:, 0:512],
                                  transpose=True)
                nc.sync.dma_start(out=xT_t[:, 4:8, :], in_=x_dram[:, 512:1024],
                                  transpose=True)
                return xT_t

            for x_dram, w_sb, b_sb, dstT in ((q_in, wq_sb, bq_sb, qT),
                                             (k_in, wk_sb, bk_sb, kT)):
                xt = load_xT(x_dram)
                for pair in range(2):
                    for st in range(4):
                        ps = projps.tile([128, 512], f32, tag="pps")
                        for c in range(8):
                            nc.tensor.matmul(
                                ps, w_sb[:, c, pair * 128:(pair + 1) * 128],
                                xt[:, c, st * 512:(st + 1) * 512],
                                start=(c == 0), stop=False)
                        # bias: column (x) ones-row outer product
                        nc.tensor.matmul(
                            ps, b_sb[:, pair * 128:(pair + 1) * 128],
                            ones1, start=False, stop=True)
                        nc.vector.tensor_copy(
                            dstT[pair][:, st * 512:(st + 1) * 512], ps)
            xt = load_xT(v_in)
            for b in range(NBLK):
                ps = projps.tile([128, 512], f32, tag="pps")
                for c in range(8):
                    nc.tensor.matmul(ps, xt[:, c, b * 128:(b + 1) * 128],
                                     wv_sb[:, c, :], start=(c == 0), stop=False)
                nc.tensor.matmul(ps, ones1[:, 0:128], bv_sb,
                                 start=False, stop=True)
                nc.vector.tensor_copy(vA[:, b, :], ps)

        # ---- main attention pools ----
        lg_ps = ctx.enter_context(tc.tile_pool(name="lgps", bufs=3, space="PSUM"))
        at_ps = ctx.enter_context(tc.tile_pool(name="atps", bufs=2, space="PSUM"))
        expnat = ctx.enter_context(tc.tile_pool(name="expnat", bufs=4))
        expT_p = ctx.enter_context(tc.tile_pool(name="expT", bufs=1))
        aw_p = ctx.enter_context(tc.tile_pool(name="aw", bufs=3))
        small = ctx.enter_context(tc.tile_pool(name="small", bufs=6))
        rec_p = ctx.enter_context(tc.tile_pool(name="rec", bufs=2))
        rb_p = ctx.enter_context(tc.tile_pool(name="rb", bufs=3))
        out_sb = ctx.enter_context(tc.tile_pool(name="outsb", bufs=2))

        def emit_fc(sqb):
            ot = out_sb.tile([128, D], f32, tag="out", name="ot")
            for st in range(2):
                ps = at_ps.tile([128, 512], f32, tag="at", name="fps")
                for c in range(HPC):
                    nc.tensor.matmul(ps, attnT[:, c, sqb * 128:(sqb + 1) * 128],
                                     wfc_sb[:, c, st * 512:(st + 1) * 512],
                                     start=(c == 0), stop=False)
                nc.tensor.matmul(ps, ones1[:, 0:128],
                                 bfc_sb[:, st * 512:(st + 1) * 512],
                                 start=False, stop=True)
                nc.vector.tensor_copy(ot[:, st * 512:(st + 1) * 512], ps)
            nc.sync.dma_start(out=out_p[sqb * 128:(sqb + 1) * 128, :], in_=ot)

        for pt in range(2):
            qs = [qT[pt][0:64, :], qT[pt][64:128, :]]
            ks = [kT[pt][0:64, :], kT[pt][64:128, :]]
            recs = [rec_p.tile([128, NBLK], f32, tag="rec", name=f"rec{s}")
                    for s in range(2)]

            # natural side, head pair packed into PE row groups
            for sqb in range(NBLK):
                sk_len = (sqb + 1) * 128 if causal else S
                ntile = (sk_len + 1023) // 1024
                ets = [expnat.tile([128, S], bf16, tag="expnat", name=f"et{s}")
                       for s in range(2)]
                parts = [small.tile([128, 2], f32, tag="parts", name=f"pt{s}")
                         for s in range(2)]
                for i in range(ntile):
                    tw = min(1024, sk_len - i * 1024)
                    pss = [lg_ps.tile([128, 1024], f32, tag="lg", name=f"lg{s}")
                           for s in range(2)]
                    pos = 0
                    while pos < tw:
                        w = min(512, tw - pos)
                        col = i * 1024 + pos
                        last = causal and (col + w == sk_len)
                        for s in range(2):
                            nc.tensor.matmul(
                                pss[s][:, pos:pos + w],
                                qs[s][:, sqb * 128:(sqb + 1) * 128],
                                ks[s][:, col:col + w],
                                start=True, stop=not last)
                        if last:
                            for s in range(2):
                                nc.tensor.matmul(
                                    pss[s][:, pos + w - 128:pos + w], ident,
                                    maskN, start=False, stop=True)
                        pos += w
                    for s in range(2):
                        nc.scalar.activation(
                            ets[s][:, i * 1024:i * 1024 + tw], pss[s][:, :tw],
                            AF.Exp, scale=SCALE, accum_out=parts[s][:, i:i + 1])
                for s in range(2):
                    if ntile > 1:
                        tsum = small.tile([128, 1], f32, tag="tsum")
                        nc.vector.reduce_sum(out=tsum, in_=parts[s][:, :ntile],
                                             axis=mybir.AxisListType.X)
                        nc.vector.reciprocal(recs[s][:, sqb:sqb + 1], tsum)
                    else:
                        nc.vector.reciprocal(recs[s][:, sqb:sqb + 1],
                                             parts[s][:, 0:1])
                    aw = aw_p.tile([128, S], f32, tag="aw")
                    nc.vector.tensor_scalar_mul(aw[:, :sk_len],
                                                ets[s][:, :sk_len],
                                                recs[s][:, sqb:sqb + 1])
                    nc.sync.dma_start(
                        out=attn_w[2 * pt + s,
                                   sqb * 128:(sqb + 1) * 128, 0:sk_len],
                        in_=aw[:, :sk_len])

            rbs = []
            for s in range(2):
                # broadcast recip over partitions via a DRAM round-trip
                nc.gpsimd.dma_start(
                    out=bass.AP(tensor=rscr.tensor, offset=(2 * pt + s) * S,
                                ap=[[1, 128], [128, NBLK]]),
                    in_=recs[s])
                rb = rb_p.tile([128, S], f32, tag="rb", name=f"rb{s}")
                nc.gpsimd.dma_start(
                    out=rb.rearrange("p (b j) -> p b j", j=128),
                    in_=bass.AP(tensor=rscr.tensor, offset=(2 * pt + s) * S,
                                ap=[[0, 128], [128, NBLK], [1, 128]]))
                rbs.append(rb)

            for s in range(2):
                h = 2 * pt + s
                rb = rbs[s]
                # transposed side: logitsT [sk, sq] -> expT
                eT = expT_p.tile([128, TRI], bf16, tag="eT", name="eT")
                for skc in range(NBLK):
                    sq0 = skc * 128 if causal else 0
                    o_c = TRI_OFF[skc]
                    pos = sq0
                    while pos < S:
                        tw = min(1024, S - pos)
                        ps = lg_ps.tile([128, 1024], f32, tag="lg", name="lgT")
                        ipos = 0
                        while ipos < tw:
                            w = min(512, tw - ipos)
                            diag = causal and (pos == sq0 and ipos == 0)
                            nc.tensor.matmul(
                                ps[:, ipos:ipos + w],
                                ks[s][:, skc * 128:(skc + 1) * 128],
                                qs[s][:, pos + ipos:pos + ipos + w],
                                start=True, stop=not diag)
                            if diag:
                                nc.tensor.matmul(ps[:, 0:128], ident, maskT,
                                                 start=False, stop=True)
                            ipos += w
                        nc.scalar.activation(
                            eT[:, o_c + pos - sq0:o_c + pos - sq0 + tw],
                            ps[:, :tw], AF.Exp, scale=SCALE)
                        pos += tw

                # attn^T [dh, sq] strips: lhsT = v chunk, rhs = expT
                for st in range(4):
                    smax = (st + 1) * 4 if causal else NBLK
                    aps = at_ps.tile([128, 512], f32, tag="at", name="aps")
                    for skc in range(smax):
                        sq0 = skc * 128 if causal else 0
                        cst = st * 512
                        lo = max(cst, sq0)
                        wdt = cst + 512 - lo
                        nc.tensor.matmul(
                            aps[:, lo - cst:512],
                            vA[:, skc, h * 128:(h + 1) * 128],
                            eT[:, TRI_OFF[skc] + lo - sq0:
                               TRI_OFF[skc] + lo - sq0 + wdt],
                            start=(skc == 0), stop=(skc == smax - 1),
                            skip_group_check=True)
                    nc.vector.tensor_mul(
                        attnT[:, h, st * 512:(st + 1) * 512], aps,
                        rb[:, st * 512:(st + 1) * 512])
                    if h == HPC - 1:
                        for sqb in range(st * 4, st * 4 + 4):
                            emit_fc(sqb)

    nc.compile()
    return nc


def _get_program(causal: bool):
    if causal not in _CACHE:
        _CACHE[causal] = _build(causal)
    return _CACHE[causal]


def kernel(q_in, k_in, v_in, mask, Wq_w, Wq_b, Wk_w, Wk_b, Wv_w, Wv_b,
           fc_w, fc_b):
    global last_result
    q_in = np.asarray(q_in, dtype=np.float32)
    k_in = np.asarray(k_in, dtype=np.float32)
    v_in = np.asarray(v_in, dtype=np.float32)
    mask = np.asarray(mask).astype(bool).reshape(S, S)
    Wq_w = np.asarray(Wq_w, dtype=np.float32)
    Wq_b = np.asarray(Wq_b, dtype=np.float32)
    Wk_w = np.asarray(Wk_w, dtype=np.float32)
    Wk_b = np.asarray(Wk_b, dtype=np.float32)
    Wv_w = np.asarray(Wv_w, dtype=np.float32)
    Wv_b = np.asarray(Wv_b, dtype=np.float32)
    fc_w = np.asarray(fc_w, dtype=np.float32)
    fc_b = np.asarray(fc_b, dtype=np.float32)

    causal_ref = np.triu(np.ones((S, S), dtype=bool), k=1)
    if np.array_equal(mask, causal_ref):
        causal = True
    elif not mask.any():
        causal = False
    else:
        raise NotImplementedError("only causal or empty masks supported")

    nc = _get_program(causal)

    qb = [_to_bf16(q_in[b]) for b in range(B)]
    kb = [_to_bf16(k_in[b]) for b in range(B)]
    vb = [_to_bf16(v_in[b]) for b in range(B)]

    in_maps = []
    for core in range(8):
        b, g = divmod(core, 2)
        wq_s = Wq_w[:, g * HPC * DQK:(g + 1) * HPC * DQK]       # [1024, 256]
        wk_s = Wk_w[:, g * HPC * DQK:(g + 1) * HPC * DQK]
        wv_s = Wv_w[:, g * HPC * DH:(g + 1) * HPC * DH]         # [1024, 512]
        wfc_s = fc_w[g * HPC * DH:(g + 1) * HPC * DH, :]        # [512, 1024]
        in_maps.append({
            "q_in": qb[b],
            "k_in": kb[b],
            "v_in": vb[b],
            "wq": _to_bf16(wq_s.reshape(8, 128, 2 * DH).transpose(1, 0, 2)),
            "wk": _to_bf16(wk_s.reshape(8, 128, 2 * DH).transpose(1, 0, 2)),
            "wv": _to_bf16(wv_s.reshape(8, 128, 4 * DH).transpose(1, 0, 2)),
            "wfc": _to_bf16(wfc_s.reshape(HPC, 128, D).transpose(1, 0, 2)),
            "bq": _to_bf16(Wq_b[g * HPC * DQK:(g + 1) * HPC * DQK].reshape(1, -1)),
            "bk": _to_bf16(Wk_b[g * HPC * DQK:(g + 1) * HPC * DQK].reshape(1, -1)),
            "bv": _to_bf16(Wv_b[g * HPC * DH:(g + 1) * HPC * DH].reshape(1, -1)),
            "bfc": _to_bf16(fc_b.reshape(1, D)),
        })

    trace = os.environ.get("KERNEL_TRACE") == "1"
    if trace:
        try:
            import antenv.axon_hooks  # noqa: F401  (wired by test harness)
        except ImportError:
            trace = False
    res = run_bass_kernel_spmd(nc, in_maps, core_ids=list(range(8)), trace=trace)
    last_result = res

    out = np.zeros((B, S, D), dtype=np.float32)
    attn = np.empty((B, H, S, S), dtype=np.float32)
    for core in range(8):
        b, g = divmod(core, 2)
        out[b] += res.results[core]["out_p"]
        attn[b, g * HPC:(g + 1) * HPC] = res.results[core]["attn_w"]
    return out, attn


# revision 23
# speedup vs baseline: 1.0665x; 1.0215x over previous
"""Trainium2 Bass kernel for BaselineAttention (B=4, S=2048, D=1024, H=8).

Sharding: 8 cores = 4 batches x 2 head-groups (4 heads each).
Each core computes, for its (batch, 4 heads):
  q/k/v projections (bf16 matmuls, fp32 accum), causal attention with
  fp32 softmax (exp on ACT with fused row-sums), attention-weight output,
  and its row-parallel slice of the final fc (partials summed on host).

Inputs/weights are cast to bf16 on the host (part of the sharding/layout
step); transposed activation layouts are produced by the DMA xbar
transpose directly from DRAM. Logits are computed in both [sq,sk] and
[sk,sq] orientations on the TensorEngine (cheaper than transposing the
softmax output on-chip); q/k head pairs share the PE array via row
groups (K=64 each).
"""

import math
import os
from contextlib import ExitStack

import ml_dtypes
import numpy as np

import concourse.bass as bass
import concourse.mybir as mybir
import concourse.tile as tile
from concourse import bacc
from concourse.bass_utils import run_bass_kernel_spmd
from concourse.masks import make_causal_mask, make_identity

B, S, D, H = 4, 2048, 1024, 8
DH = D // H            # 128 (v head dim)
DQK = (D // 2) // H    # 64  (q/k head dim)
HPC = H // 2           # 4 heads per core
NBLK = S // 128        # 16
SCALE = 1.0 / math.sqrt(DH)
NEG = -1e9

bf16 = mybir.dt.bfloat16
f32 = mybir.dt.float32
AF = mybir.ActivationFunctionType

_CACHE = {}
last_result = None  # stashed BassKernelResults when KERNEL_TRACE=1


def _to_bf16(x):
    """Round-to-nearest-even float32 -> bfloat16, numpy only."""
    u = np.ascontiguousarray(x, dtype=np.float32).view(np.uint32)
    r = (u >> 16) & np.uint32(1)
    out = ((u + np.uint32(0x7FFF) + r) >> 16).astype(np.uint16)
    return out.view(ml_dtypes.bfloat16)


def _tri_offsets(causal):
    """Column offsets of each sk-chunk's row inside the triangular expT tile."""
    offs, tot = [], 0
    for c in range(NBLK):
        offs.append(tot)
        tot += (S - c * 128) if causal else S
    return offs, tot


def _build(causal: bool):
    nc = bacc.Bacc("TRN2", target_bir_lowering=False, debug=False)

    q_in = nc.dram_tensor("q_in", [S, D], bf16, kind="ExternalInput").ap()
    k_in = nc.dram_tensor("k_in", [S, D], bf16, kind="ExternalInput").ap()
    v_in = nc.dram_tensor("v_in", [S, D], bf16, kind="ExternalInput").ap()
    # weights pre-arranged host-side: [partition, k-chunk, out-cols]
    wq = nc.dram_tensor("wq", [128, 8, 2 * DH], bf16, kind="ExternalInput").ap()
    wk = nc.dram_tensor("wk", [128, 8, 2 * DH], bf16, kind="ExternalInput").ap()
    wv = nc.dram_tensor("wv", [128, 8, 4 * DH], bf16, kind="ExternalInput").ap()
    wfc = nc.dram_tensor("wfc", [128, HPC, D], bf16, kind="ExternalInput").ap()
    # bias rows (added via K=1 rank-1 matmuls into the psum accumulation)
    bq = nc.dram_tensor("bq", [1, 2 * DH], bf16, kind="ExternalInput").ap()
    bk = nc.dram_tensor("bk", [1, 2 * DH], bf16, kind="ExternalInput").ap()
    bv = nc.dram_tensor("bv", [1, 4 * DH], bf16, kind="ExternalInput").ap()
    bfc = nc.dram_tensor("bfc", [1, D], bf16, kind="ExternalInput").ap()

    rscr = nc.dram_tensor("rscr", [HPC, S], f32).ap()
    attn_w = nc.dram_tensor("attn_w", [HPC, S, S], f32, kind="ExternalOutput").ap()
    out_p = nc.dram_tensor("out_p", [S, D], f32, kind="ExternalOutput").ap()

    TRI_OFF, TRI = _tri_offsets(causal)

    with tile.TileContext(nc) as tc, ExitStack() as ctx:
        singles = ctx.enter_context(tc.tile_pool(name="singles", bufs=1))
        ident = singles.tile([128, 128], bf16)
        make_identity(nc, ident)
        if causal:
            maskN = singles.tile([128, 128], bf16)
            make_causal_mask(nc, maskN, mask_val=NEG)  # fill where col > row
            maskT = singles.tile([128, 128], bf16)     # fill where row > col
            nc.gpsimd.memset(maskT, 0.0)
            nc.gpsimd.affine_select(
                out=maskT, in_=maskT, compare_op=mybir.AluOpType.is_ge,
                fill=NEG, base=0, pattern=[[1, 128]], channel_multiplier=-1,
            )
        # weights via SWDGE so the HWDGE xbar stays in transpose mode
        wq_sb = singles.tile([128, 8, 2 * DH], bf16)
        nc.gpsimd.dma_start(out=wq_sb, in_=wq)
        wk_sb = singles.tile([128, 8, 2 * DH], bf16)
        nc.gpsimd.dma_start(out=wk_sb, in_=wk)
        wv_sb = singles.tile([128, 8, 4 * DH], bf16)
        nc.gpsimd.dma_start(out=wv_sb, in_=wv)
        wfc_sb = singles.tile([128, HPC, D], bf16)
        nc.gpsimd.dma_start(out=wfc_sb, in_=wfc)
        bq_sb = singles.tile([1, 2 * DH], bf16)
        nc.gpsimd.dma_start(out=bq_sb, in_=bq)
        bk_sb = singles.tile([1, 2 * DH], bf16)
        nc.gpsimd.dma_start(out=bk_sb, in_=bk)
        bv_sb = singles.tile([1, 4 * DH], bf16)
        nc.gpsimd.dma_start(out=bv_sb, in_=bv)
        bfc_sb = singles.tile([1, D], bf16)
        nc.gpsimd.dma_start(out=bfc_sb, in_=bfc)
        ones1 = singles.tile([1, 512], bf16)
        nc.vector.memset(ones1, 1.0)

        acts = ctx.enter_context(tc.tile_pool(name="acts", bufs=1))
        qT = [acts.tile([128, S], bf16, tag=f"qT{i}", name=f"qT{i}")
              for i in range(2)]
        kT = [acts.tile([128, S], bf16, tag=f"kT{i}", name=f"kT{i}")
              for i in range(2)]
        vA = acts.tile([128, NBLK, 4 * DH], bf16, tag="v")
        attnT = acts.tile([128, HPC, S], bf16, tag="attnT")

        # ---- transpose-load inputs (DMA xbar from DRAM bf16), project ----
        with tc.tile_pool(name="xT", bufs=2) as xTp, \
             tc.tile_pool(name="projps", bufs=2, space="PSUM") as projps:

            def load_xT(x_dram):
                xT_t = xTp.tile([128, 8, S], bf16, tag="xT")
                nc.sync.dma_start(out=xT_t[:, 0:4, :], in_=x_dram[:, 0:512],
                                  transpose=True)
                nc.sync.dma_start(out=xT_t[:, 4:8, :], in_=x_dram[:, 512:1024],
                                  transpose=True)
                return xT_t

            for x_dram, w_sb, b_sb, dstT in ((q_in, wq_sb, bq_sb, qT),
                                             (k_in, wk_sb, bk_sb, kT)):
                xt = load_xT(x_dram)
                for pair in range(2):
                    for st in range(4):
                        ps = projps.tile([128, 512], f32, tag="pps")
                        for c in range(8):
                            nc.tensor.matmul(
                                ps, w_sb[:, c, pair * 128:(pair + 1) * 128],
                                xt[:, c, st * 512:(st + 1) * 512],
                                start=(c == 0), stop=False)
                        # bias: column (x) ones-row outer product
                        nc.tensor.matmul(
                            ps, b_sb[:, pair * 128:(pair + 1) * 128],
                            ones1, start=False, stop=True)
                        nc.vector.tensor_copy(
                            dstT[pair][:, st * 512:(st + 1) * 512], ps)
            xt = load_xT(v_in)
            for b in range(NBLK):
                ps = projps.tile([128, 512], f32, tag="pps")
                for c in range(8):
                    nc.tensor.matmul(ps, xt[:, c, b * 128:(b + 1) * 128],
                                     wv_sb[:, c, :], start=(c == 0), stop=False)
                nc.tensor.matmul(ps, ones1[:, 0:128], bv_sb,
                                 start=False, stop=True)
                nc.vector.tensor_copy(vA[:, b, :], ps)

        # ---- main attention pools ----
        lg_ps = ctx.enter_context(tc.tile_pool(name="lgps", bufs=3, space="PSUM"))
        at_ps = ctx.enter_context(tc.tile_pool(name="atps", bufs=2, space="PSUM"))
        expnat = ctx.enter_context(tc.tile_pool(name="expnat", bufs=4))
        expT_p = ctx.enter_context(tc.tile_pool(name="expT", bufs=1))
        aw_p = ctx.enter_context(tc.tile_pool(name="aw", bufs=3))
        small = ctx.enter_context(tc.tile_pool(name="small", bufs=6))
        rec_p = ctx.enter_context(tc.tile_pool(name="rec", bufs=2))
        rb_p = ctx.enter_context(tc.tile_pool(name="rb", bufs=3))
        out_sb = ctx.enter_context(tc.tile_pool(name="outsb", bufs=2))

        def emit_fc(sqb):
            ot = out_sb.tile([128, D], f32, tag="out", name="ot")
            for st in range(2):
                ps = at_ps.tile([128, 512], f32, tag="at", name="fps")
                for c in range(HPC):
                    nc.tensor.matmul(ps, attnT[:, c, sqb * 128:(sqb + 1) * 128],
                                     wfc_sb[:, c, st * 512:(st + 1) * 512],
                                     start=(c == 0), stop=False)
                nc.tensor.matmul(ps, ones1[:, 0:128],
                                 bfc_sb[:, st * 512:(st + 1) * 512],
                                 start=False, stop=True)
                nc.vector.tensor_copy(ot[:, st * 512:(st + 1) * 512], ps)
            nc.sync.dma_start(out=out_p[sqb * 128:(sqb + 1) * 128, :], in_=ot)

        for pt in range(2):
            qs = [qT[pt][0:64, :], qT[pt][64:128, :]]
            ks = [kT[pt][0:64, :], kT[pt][64:128, :]]
            recs = [rec_p.tile([128, NBLK], f32, tag="rec", name=f"rec{s}")
                    for s in range(2)]

            # natural side, head pair packed into PE row groups
            for sqb in range(NBLK):
                sk_len = (sqb + 1) * 128 if causal else S
                ntile = (sk_len + 1023) // 1024
                ets = [expnat.tile([128, S], bf16, tag="expnat", name=f"et{s}")
                       for s in range(2)]
                parts = [small.tile([128, 2], f32, tag="parts", name=f"pt{s}")
                         for s in range(2)]
                for i in range(ntile):
                    tw = min(1024, sk_len - i * 1024)
                    pss = [lg_ps.tile([128, 1024], f32, tag="lg", name=f"lg{s}")
                           for s in range(2)]
                    pos = 0
                    while pos < tw:
                        w = min(512, tw - pos)
                        col = i * 1024 + pos
                        last = causal and (col + w == sk_len)
                        for s in range(2):
                            nc.tensor.matmul(
                                pss[s][:, pos:pos + w],
                                qs[s][:, sqb * 128:(sqb + 1) * 128],
                                ks[s][:, col:col + w],
                                start=True, stop=not last)
                        if last:
                            for s in range(2):
                                nc.tensor.matmul(
                                    pss[s][:, pos + w - 128:pos + w], ident,
                                    maskN, start=False, stop=True)
                        pos += w
                    for s in range(2):
                        nc.scalar.activation(
                            ets[s][:, i * 1024:i * 1024 + tw], pss[s][:, :tw],
                            AF.Exp, scale=SCALE, accum_out=parts[s][:, i:i + 1])
                for s in range(2):
                    if ntile > 1:
                        tsum = small.tile([128, 1], f32, tag="tsum")
                        nc.vector.reduce_sum(out=tsum, in_=parts[s][:, :ntile],
                                             axis=mybir.AxisListType.X)
                        nc.vector.reciprocal(recs[s][:, sqb:sqb + 1], tsum)
                    else:
                        nc.vector.reciprocal(recs[s][:, sqb:sqb + 1],
                                             parts[s][:, 0:1])
                    aw = aw_p.tile([128, S], f32, tag="aw")
                    nc.vector.tensor_scalar_mul(aw[:, :sk_len],
                                                ets[s][:, :sk_len],
                                                recs[s][:, sqb:sqb + 1])
                    nc.sync.dma_start(
                        out=attn_w[2 * pt + s,
                                   sqb * 128:(sqb + 1) * 128, 0:sk_len],
                        in_=aw[:, :sk_len])

            rbs = []
            for s in range(2):
                # broadcast recip over partitions via a DRAM round-trip
                nc.sync.dma_start(
                    out=bass.AP(tensor=rscr.tensor, offset=(2 * pt + s) * S,
                                ap=[[1, 128], [128, NBLK]]),
                    in_=recs[s])
                rb = rb_p.tile([128, S], f32, tag="rb", name=f"rb{s}")
                nc.sync.dma_start(
                    out=rb.rearrange("p (b j) -> p b j", j=128),
                    in_=bass.AP(tensor=rscr.tensor, offset=(2 * pt + s) * S,
                                ap=[[0, 128], [128, NBLK], [1, 128]]))
                rbs.append(rb)

            for s in range(2):
                h = 2 * pt + s
                rb = rbs[s]
                # transposed side: logitsT [sk, sq] -> expT
                eT = expT_p.tile([128, TRI], bf16, tag="eT", name="eT")
                for skc in range(NBLK):
                    sq0 = skc * 128 if causal else 0
                    o_c = TRI_OFF[skc]
                    pos = sq0
                    while pos < S:
                        tw = min(1024, S - pos)
                        ps = lg_ps.tile([128, 1024], f32, tag="lg", name="lgT")
                        ipos = 0
                        while ipos < tw:
                            w = min(512, tw - ipos)
                            diag = causal and (pos == sq0 and ipos == 0)
                            nc.tensor.matmul(
                                ps[:, ipos:ipos + w],
                                ks[s][:, skc * 128:(skc + 1) * 128],
                                qs[s][:, pos + ipos:pos + ipos + w],
                                start=True, stop=not diag)
                            if diag:
                                nc.tensor.matmul(ps[:, 0:128], ident, maskT,
                                                 start=False, stop=True)
                            ipos += w
                        nc.scalar.activation(
                            eT[:, o_c + pos - sq0:o_c + pos - sq0 + tw],
                            ps[:, :tw], AF.Exp, scale=SCALE)
                        pos += tw

                # attn^T [dh, sq] strips: lhsT = v chunk, rhs = expT
                for st in range(4):
                    smax = (st + 1) * 4 if causal else NBLK
                    aps = at_ps.tile([128, 512], f32, tag="at", name="aps")
                    for skc in range(smax):
                        sq0 = skc * 128 if causal else 0
                        cst = st * 512
                        lo = max(cst, sq0)
                        wdt = cst + 512 - lo
                        nc.tensor.matmul(
                            aps[:, lo - cst:512],
                            vA[:, skc, h * 128:(h + 1) * 128],
                            eT[:, TRI_OFF[skc] + lo - sq0:
                               TRI_OFF[skc] + lo - sq0 + wdt],
                            start=(skc == 0), stop=(skc == smax - 1),
                            skip_group_check=True)
                    nc.vector.tensor_mul(
                        attnT[:, h, st * 512:(st + 1) * 512], aps,
                        rb[:, st * 512:(st + 1) * 512])
                    if h == HPC - 1:
                        for sqb in range(st * 4, st * 4 + 4):
                            emit_fc(sqb)

    nc.compile()
    return nc


def _get_program(causal: bool):
    if causal not in _CACHE:
        _CACHE[causal] = _build(causal)
    return _CACHE[causal]


def kernel(q_in, k_in, v_in, mask, Wq_w, Wq_b, Wk_w, Wk_b, Wv_w, Wv_b,
           fc_w, fc_b):
    global last_result
    q_in = np.asarray(q_in, dtype=np.float32)
    k_in = np.asarray(k_in, dtype=np.float32)
    v_in = np.asarray(v_in, dtype=np.float32)
    mask = np.asarray(mask).astype(bool).reshape(S, S)
    Wq_w = np.asarray(Wq_w, dtype=np.float32)
    Wq_b = np.asarray(Wq_b, dtype=np.float32)
    Wk_w = np.asarray(Wk_w, dtype=np.float32)
    Wk_b = np.asarray(Wk_b, dtype=np.float32)
    Wv_w = np.asarray(Wv_w, dtype=np.float32)
    Wv_b = np.asarray(Wv_b, dtype=np.float32)
    fc_w = np.asarray(fc_w, dtype=np.float32)
    fc_b = np.asarray(fc_b, dtype=np.float32)

    causal_ref = np.triu(np.ones((S, S), dtype=bool), k=1)
    if np.array_equal(mask, causal_ref):
        causal = True
    elif not mask.any():
        causal = False
    else:
        raise NotImplementedError("only causal or empty masks supported")

    nc = _get_program(causal)

    qb = [_to_bf16(q_in[b]) for b in range(B)]
    kb = [_to_bf16(k_in[b]) for b in range(B)]
    vb = [_to_bf16(v_in[b]) for b in range(B)]

    in_maps = []
    for core in range(8):
        b, g = divmod(core, 2)
        wq_s = Wq_w[:, g * HPC * DQK:(g + 1) * HPC * DQK]       # [1024, 256]
        wk_s = Wk_w[:, g * HPC * DQK:(g + 1) * HPC * DQK]
        wv_s = Wv_w[:, g * HPC * DH:(g + 1) * HPC * DH]         # [1024, 512]
        wfc_s = fc_w[g * HPC * DH:(g + 1) * HPC * DH, :]        # [512, 1024]
        in_maps.append({
            "q_in": qb[b],
            "k_in": kb[b],
            "v_in": vb[b],
            "wq": _to_bf16(wq_s.reshape(8, 128, 2 * DH).transpose(1, 0, 2)),
            "wk": _to_bf16(wk_s.reshape(8, 128, 2 * DH).transpose(1, 0, 2)),
            "wv": _to_bf16(wv_s.reshape(8, 128, 4 * DH).transpose(1, 0, 2)),
            "wfc": _to_bf16(wfc_s.reshape(HPC, 128, D).transpose(1, 0, 2)),
            "bq": _to_bf16(Wq_b[g * HPC * DQK:(g + 1) * HPC * DQK].reshape(1, -1)),
            "bk": _to_bf16(Wk_b[g * HPC * DQK:(g + 1) * HPC * DQK].reshape(1, -1)),
            "bv": _to_bf16(Wv_b[g * HPC * DH:(g + 1) * HPC * DH].reshape(1, -1)),
            "bfc": _to_bf16(fc_b.reshape(1, D)),
        })

    trace = os.environ.get("KERNEL_TRACE") == "1"
    if trace:
        try:
            import antenv.axon_hooks  # noqa: F401  (wired by test harness)
        except ImportError:
            trace = False
    res = run_bass_kernel_spmd(nc, in_maps, core_ids=list(range(8)), trace=trace)
    last_result = res

    out = np.zeros((B, S, D), dtype=np.float32)
    attn = np.empty((B, H, S, S), dtype=np.float32)
    for core in range(8):
        b, g = divmod(core, 2)
        out[b] += res.results[core]["out_p"]
        attn[b, g * HPC:(g + 1) * HPC] = res.results[core]["attn_w"]
    return out, attn
